# revision 45
# baseline (speedup 1.0000x reference)
"""Trainium2 Bass kernel for CodebookRemapReadout.

Data-parallel over n across 8 NeuronCores. Each core handles 64 queries.

Math (per query n):
  q_proj      = q @ Wq.T
  cwu[c]      = exp(q_proj . cb_c / sqrt(D))          (unnormalized)
  Zq          = sum_c cwu[c]
  E[c, s]     = exp(K[n,s] . cb_c / sqrt(D))          (unnormalized)
  Z[s]        = sum_c E[c, s]
  W[s]        = sum_c cwu[c] E[c, s]
  eff[s]      = W[s] / (Zq * Z[s])                    (== softmax-contracted weights)
  summary     = eff @ V[n]
  logits      = summary @ Wc.T + bc
  recon       = summary @ Wr.T + br

Layout strategy: keep the feature/codebook dim on SBUF partitions everywhere
(host pre-transposes K, cb, Wq, Wc, Wr into fp16), so the big [NS, C] logits
matmul, the Z/W reduction and the final projections are all plain PE matmuls
with zero on-device input transposes.
"""

import math

import numpy as np

N, S, D, C, CLASSES = 512, 256, 512, 1024, 1000
NCORES = 8
NSH = N // NCORES          # 64 queries per core
P = 128                    # partitions
NS = NSH * S               # 16384 ns-columns per core
CHUNK = 512                # ns-columns per chunk (= 2 queries)
NCHUNK = NS // CHUNK       # 32
GROUP = 32                 # queries per summary group (32-aligned partition starts)
NGROUP = NSH // GROUP      # 4
SCALE = 1.0 / math.sqrt(D)

_cache = {}


def _patch_ldw_opt():
    """Flip walrus's --enable-ldw-opt to true (dedupe/overlap weight loads).
    Env-gated for A/B: set KERNEL_NO_LDW_OPT=1 to keep the default."""
    import os

    # walrus rejects Tile-emitted InstLdweights under ldw-opt ("InstLdweights
    # is not compatible with LDW optimization"), so this stays off unless
    # explicitly requested for experiments.
    if not os.environ.get("KERNEL_LDW_OPT"):
        return
    import concourse.bass_utils as bu

    if getattr(bu, "_ldw_opt_patched", False):
        return
    orig = bu.run_command
    def _run(cmd, *a, **kw):
        cmd = [
            "--enable-ldw-opt=true" if c == "--enable-ldw-opt=false" else c
            for c in cmd
        ]
        return orig(cmd, *a, **kw)
    bu.run_command = _run
    bu._ldw_opt_patched = True


def _dedupe_ldweights(nc):
    """Delete a Ldweights instruction when the immediately-previous Ldweights
    in the same block loads identical weights (same memref/offset/ap/mode)
    and the candidate carries no semaphore waits/updates. The PE array keeps
    the stationary operand across matmuls, so the reload is pure overhead
    (verified bit-exact on hardware). All matmuls here are fp16, which lower
    to explicit Ldweights+Matmult pairs — fp32 self-loading matmuls would
    clobber the array and are not used."""

    def sig_of(ins):
        ap0 = ins.ins[0]
        return (
            str(getattr(ap0, "memref", None)),
            getattr(ap0, "offset", None),
            str(getattr(ap0, "ap", None)),
            str(getattr(ins, "tile_position", None)),
            str(getattr(ins, "perf_mode", None)),
            str(getattr(ins, "is_transpose", None)),
        )

    removed = 0
    for f in nc.m.functions:
        for blk in f.blocks:
            last_sig = None
            drop = []
            for idx, ins in enumerate(blk.instructions):
                tn = type(ins).__name__
                if tn == "InstLdweights":
                    sig = sig_of(ins)
                    si = ins.sync_info
                    clean = si is None or (
                        len(si.on_wait) == 0 and len(si.on_update) == 0
                    )
                    if sig == last_sig and clean:
                        drop.append(idx)
                    else:
                        last_sig = sig
                elif tn == "InstMatmult":
                    pass  # fp16 matmults do not touch the weight registers
                elif ins.engine is not None and str(ins.engine) == "EngineType.PE":
                    # unknown PE instruction: be conservative
                    last_sig = None
            for idx in reversed(drop):
                del blk.instructions[idx]
            removed += len(drop)
    return removed


def _build_program():
    import concourse.bass as bass
    import concourse.bacc as bacc
    import concourse.tile as tile
    import concourse.mybir as mybir

    f16 = mybir.dt.float16
    f32 = mybir.dt.float32
    f32r = mybir.dt.float32r
    Exp = mybir.ActivationFunctionType.Exp

    nc = bacc.Bacc("TRN2", target_bir_lowering=False, debug=False)

    # ---- per-core DRAM I/O ----
    qT_d = nc.dram_tensor("qT", [D, NSH], f16, kind="ExternalInput")
    kT_d = nc.dram_tensor("kT", [D, NS], f16, kind="ExternalInput")
    v_d = nc.dram_tensor("v", [NSH, S, D], f16, kind="ExternalInput")
    cbT_d = nc.dram_tensor("cbT", [D, C], f16, kind="ExternalInput")
    wqT_d = nc.dram_tensor("wqT", [D, D], f16, kind="ExternalInput")
    wcT_d = nc.dram_tensor("wcT", [D, CLASSES], f16, kind="ExternalInput")
    wrT_d = nc.dram_tensor("wrT", [D, D], f16, kind="ExternalInput")
    bc_d = nc.dram_tensor("bc", [1, CLASSES], f16, kind="ExternalInput")
    br_d = nc.dram_tensor("br", [1, D], f16, kind="ExternalInput")
    ones_d = nc.dram_tensor("onesv", [1, P], f16, kind="ExternalInput")

    logits_d = nc.dram_tensor("logits", [NSH, CLASSES], f32, kind="ExternalOutput")
    recon_d = nc.dram_tensor("recon", [NSH, D], f32, kind="ExternalOutput")
    eff_d = nc.dram_tensor("eff", [NSH, S], f32, kind="ExternalOutput")

    DK = D // P  # 4 contraction chunks
    CT = C // P  # 8 codebook tiles

    with tile.TileContext(nc) as tc:
        with (
            tc.tile_pool(name="resident", bufs=1) as rp,
            tc.tile_pool(name="kt", bufs=16) as ktp,
            tc.tile_pool(name="et", bufs=20) as etp,
            tc.tile_pool(name="vt", bufs=6) as vtp,
            tc.tile_pool(name="small", bufs=3) as smp,
            tc.tile_pool(name="psL", bufs=3, space="PSUM") as psL,
            tc.tile_pool(name="psZW", bufs=2, space="PSUM") as psZW,
            tc.tile_pool(name="psBC", bufs=2, space="PSUM") as psBC,
            tc.tile_pool(name="psS", bufs=1, space="PSUM") as psS,
        ):
            # ---------- resident loads ----------
            cbT = [rp.tile([P, C], f16, tag=f"cbT{i}", name=f"cbT{i}") for i in range(DK)]
            for i in range(DK):
                nc.sync.dma_start(cbT[i][:], cbT_d[i * P : (i + 1) * P, :])
            # views of the codebook weights as [128, 128] lhsT slices
            cbW = [
                [cbT[i][:, ct * P : (ct + 1) * P] for ct in range(C // P)]
                for i in range(DK)
            ]
            wqT = [rp.tile([P, D], f16, tag=f"wqT{i}", name=f"wqT{i}") for i in range(DK)]
            for i in range(DK):
                nc.sync.dma_start(wqT[i][:], wqT_d[i * P : (i + 1) * P, :])
            wcT = [rp.tile([P, CLASSES], f16, tag=f"wcT{i}", name=f"wcT{i}") for i in range(DK)]
            for i in range(DK):
                nc.sync.dma_start(wcT[i][:], wcT_d[i * P : (i + 1) * P, :])
            wrT = [rp.tile([P, D], f16, tag=f"wrT{i}", name=f"wrT{i}") for i in range(DK)]
            for i in range(DK):
                nc.sync.dma_start(wrT[i][:], wrT_d[i * P : (i + 1) * P, :])
            qT = [rp.tile([P, NSH], f16, tag=f"qT{i}", name=f"qT{i}") for i in range(DK)]
            for i in range(DK):
                nc.sync.dma_start(qT[i][:], qT_d[i * P : (i + 1) * P, :])
            bc_sb = rp.tile([1, CLASSES], f16, tag="bc", name="bc")
            nc.sync.dma_start(bc_sb[:], bc_d[:])
            br_sb = rp.tile([1, D], f16, tag="br", name="br")
            nc.sync.dma_start(br_sb[:], br_d[:])
            ones_sb = rp.tile([1, P], f16, tag="ones", name="ones")
            nc.sync.dma_start(ones_sb[:], ones_d[:])

            # ---------- query path ----------
            # q_projT [e, n] = Wq @ q.T  (4 e-tiles)
            qp = [rp.tile([P, NSH], f16, tag=f"qp{e}", name=f"qp{e}") for e in range(DK)]
            for e in range(DK):
                ps = psS.tile([P, NSH], f32, tag="s", name="qps")
                for dk in range(DK):
                    nc.tensor.matmul(
                        ps[:],
                        wqT[dk][:, e * P : (e + 1) * P],
                        qT[dk][:],
                        start=(dk == 0),
                        stop=(dk == DK - 1),
                    )
                nc.vector.tensor_copy(qp[e][:], ps[:])

            # cwu^T [c, n] tiles with an extra ones column (col NSH): used as
            # the fused Z/W reduction weights.
            cwu = [rp.tile([P, NSH + 1], f16, tag=f"cwu{ct}", name=f"cwu{ct}") for ct in range(CT)]
            for ct in range(CT):
                ps = psS.tile([P, NSH], f32, tag="s", name="qps")
                for ek in range(DK):
                    nc.tensor.matmul(
                        ps[:],
                        cbW[ek][ct][:],
                        qp[ek][:],
                        start=(ek == 0),
                        stop=(ek == DK - 1),
                    )
                nc.scalar.activation(cwu[ct][:, 0:NSH], ps[:], Exp, scale=SCALE)
                nc.vector.memset(cwu[ct][:, NSH : NSH + 1], 1.0)

            # Zq [1, n] then Rq = 1/Zq (fp16 row at partition 0)
            zq = psS.tile([1, NSH], f32, tag="s", name="zq")
            for ct in range(CT):
                nc.tensor.matmul(
                    zq[:],
                    cwu[ct][:, NSH : NSH + 1],
                    cwu[ct][:, 0:NSH],
                    start=(ct == 0),
                    stop=(ct == CT - 1),
                )
            rq32 = rp.tile([P, NSH], f32, tag="rq32", name="rq32")
            nc.vector.reciprocal(rq32[0:1, :], zq[:])
            rq16 = rp.tile([P, NSH], f16, tag="rq16", name="rq16")
            nc.vector.tensor_copy(rq16[0:1, :], rq32[0:1, :])

            # ---------- persistent staging ----------
            estg = rp.tile([P, S], f16, tag="estg", name="estg")         # eff fp16 [n, s]
            sstg = rp.tile([NSH, D], f16, tag="sstg", name="sstg")       # summary fp16 [n, d]

            # ---------- main loop: chunk PAIRS share stationary weights ----------
            # Per pair, the two chunks' matmuls are interleaved (dk-major) so
            # consecutive PE matmuls use identical weights; the post-pass
            # below then deletes the redundant Ldweights.
            pending = []
            for pr in range(NCHUNK // 2):
                kt2 = []
                for cc in range(2):
                    ch = 2 * pr + cc
                    ktc = [
                        ktp.tile([P, CHUNK], f16, tag="kt", name=f"kt{cc}_{i}")
                        for i in range(DK)
                    ]
                    for i in range(DK):
                        nc.sync.dma_start(
                            ktc[i][:],
                            kT_d[i * P : (i + 1) * P, ch * CHUNK : (ch + 1) * CHUNK],
                        )
                    kt2.append(ktc)
                # logits tiles -> exp
                et2 = [[], []]
                for ct in range(CT):
                    L2 = [
                        psL.tile([P, CHUNK], f32, tag="L", name=f"L{cc}")
                        for cc in range(2)
                    ]
                    for dk in range(DK):
                        for cc in range(2):
                            nc.tensor.matmul(
                                L2[cc][:],
                                cbW[dk][ct][:],
                                kt2[cc][dk][:],
                                start=(dk == 0),
                                stop=(dk == DK - 1),
                            )
                    for cc in range(2):
                        e_t = etp.tile([P, CHUNK], f16, tag="E", name="E")
                        nc.scalar.activation(e_t[:], L2[cc][:], Exp, scale=SCALE)
                        et2[cc].append(e_t)
                # deferred eff chains of the previous pair go here: their PE
                # matmuls sit behind this pair's logits matmuls, and before
                # this pair's Z/W matmuls (whose PSUM slots they release).
                def eff_chain(zw, n0):
                    # rz = 1/Z on row NSH, hop to partition 0
                    rz32 = smp.tile([P, CHUNK], f32, tag="rz32", name="rz32")
                    nc.vector.reciprocal(rz32[NSH : NSH + 1, :], zw[NSH : NSH + 1, :])
                    rz16 = smp.tile([P, CHUNK], f16, tag="rz16", name="rz16")
                    nc.vector.tensor_copy(
                        rz16[NSH : NSH + 1, :], rz32[NSH : NSH + 1, :]
                    )
                    rz0 = smp.tile([P, CHUNK], f16, tag="rz0", name="rz0")
                    nc.sync.dma_start(rz0[0:1, :], rz16[NSH : NSH + 1, :])
                    # bcast[n, col] = Rq[n] * rz[col]
                    bc_ps = psBC.tile([NSH, CHUNK], f32, tag="bc", name="bc_ps")
                    nc.tensor.matmul(
                        bc_ps[:], rq16[0:1, 0:NSH], rz0[0:1, :], start=True, stop=True
                    )
                    # eff rows (fp32): multiply the whole 64-row block (engines
                    # need 32-aligned partition starts); rows n0, n0+1 valid.
                    # DVE reads at most one PSUM operand: stage bcast in SBUF.
                    bcs = smp.tile([P, CHUNK], f32, tag="bcs", name="bcs")
                    nc.vector.tensor_copy(bcs[0:NSH, :], bc_ps[:])
                    effc = smp.tile([P, CHUNK], f32, tag="effc", name="effc")
                    nc.vector.tensor_mul(effc[0:NSH, :], zw[0:NSH, :], bcs[0:NSH, :])
                    # fp16 staging ([n, s]); SWDGE DMA casts f32->f16
                    nc.gpsimd.dma_start(estg[n0 : n0 + 1, :], effc[n0 : n0 + 1, 0:S])
                    nc.gpsimd.dma_start(
                        estg[n0 + 1 : n0 + 2, :], effc[n0 + 1 : n0 + 2, S:]
                    )

                for args in pending:
                    eff_chain(*args)
                pending = []

                # fused Z/W: rows 0..NSH-1 = W per local query, row NSH = Z
                # (ct-major, chunk-inner so consecutive matmuls share weights)
                zw2 = [
                    psZW.tile([NSH + 1, CHUNK], f32, tag="zw", name=f"zw{cc}")
                    for cc in range(2)
                ]
                for ct in range(CT):
                    for cc in range(2):
                        nc.tensor.matmul(
                            zw2[cc][:],
                            cwu[ct][:, 0 : NSH + 1],
                            et2[cc][ct][:],
                            start=(ct == 0),
                            stop=(ct == CT - 1),
                        )
                for cc in range(2):
                    pending.append((zw2[cc], 2 * (2 * pr + cc)))
                if pr == NCHUNK // 2 - 1:
                    for args in pending:
                        eff_chain(*args)
                    pending = []

                # ---------- summary for a finished group ----------
                # deferred one pair past the group's last chunk so the
                # transpose + matmuls never stall the PE pipeline
                PPG = NCHUNK // 2 // NGROUP  # pairs per group
                if (pr >= PPG and (pr - PPG) % PPG == 0) or pr == NCHUNK // 2 - 1:
                    g = (pr - PPG) // PPG if pr != NCHUNK // 2 - 1 else NGROUP - 1
                    gn = g * GROUP
                    efft_a = smp.tile([P, GROUP], f16, tag="efft_a", name="efft_a")
                    nc.sync.dma_start_transpose(
                        out=efft_a[:], in_=estg[gn : gn + GROUP, 0:P]
                    )
                    efft_b = smp.tile([P, GROUP], f16, tag="efft_b", name="efft_b")
                    nc.sync.dma_start_transpose(
                        out=efft_b[:], in_=estg[gn : gn + GROUP, P:S]
                    )
                    for j in range(GROUP):
                        n = gn + j
                        v_t = vtp.tile([P, 2, D], f16, tag="v", name="v_t")
                        nc.sync.dma_start(
                            v_t[:], v_d[n].rearrange("(a p) d -> p a d", p=P)
                        )
                        s_ps = psS.tile([1, D], f32, tag="s", name="s_ps")
                        nc.tensor.matmul(
                            s_ps[:],
                            efft_a[:, j : j + 1],
                            v_t[:, 0, :],
                            start=True,
                            stop=False,
                        )
                        nc.tensor.matmul(
                            s_ps[:],
                            efft_b[:, j : j + 1],
                            v_t[:, 1, :],
                            start=False,
                            stop=True,
                        )
                        srow = smp.tile([P, D], f16, tag="srow", name="srow")
                        if j % 2 == 0:
                            nc.vector.tensor_copy(srow[0:1, :], s_ps[:])
                        else:
                            nc.scalar.copy(srow[0:1, :], s_ps[:])
                        nc.sync.dma_start(sstg[n : n + 1, :], srow[0:1, :])

            # eff output in one shot (SWDGE casts the staged fp16 back to f32)
            nc.gpsimd.dma_start(eff_d[:], estg[0:NSH, :])

            # ---------- final projections ----------
            st = [rp.tile([P, NSH], f16, tag=f"st{i}", name=f"st{i}") for i in range(DK)]
            for i in range(DK):
                nc.sync.dma_start_transpose(
                    out=st[i][:], in_=sstg[:, i * P : (i + 1) * P]
                )
            # logits = summary @ Wc.T + bc   (two 500-wide halves)
            H = CLASSES // 2
            for h in range(2):
                lg = psBC.tile([NSH, H], f32, tag="bc", name="lg")
                nc.tensor.matmul(
                    lg[:],
                    ones_sb[0:1, 0:NSH],
                    bc_sb[0:1, h * H : (h + 1) * H],
                    start=True,
                    stop=False,
                )
                for dk in range(DK):
                    nc.tensor.matmul(
                        lg[:],
                        st[dk][:, 0:NSH],
                        wcT[dk][:, h * H : (h + 1) * H],
                        start=False,
                        stop=(dk == DK - 1),
                    )
                lg_sb = smp.tile([P, H], f32, tag="lg_sb", name="lg_sb")
                nc.scalar.copy(lg_sb[0:NSH, :], lg[:])
                nc.sync.dma_start(logits_d[:, h * H : (h + 1) * H], lg_sb[0:NSH, :])
            # recon = summary @ Wr.T + br
            rc = psBC.tile([NSH, D], f32, tag="bc", name="rc")
            nc.tensor.matmul(
                rc[:], ones_sb[0:1, 0:NSH], br_sb[0:1, :], start=True, stop=False
            )
            for dk in range(DK):
                nc.tensor.matmul(
                    rc[:],
                    st[dk][:, 0:NSH],
                    wrT[dk][:, :],
                    start=False,
                    stop=(dk == DK - 1),
                )
            rc_sb = smp.tile([P, D], f32, tag="rc_sb", name="rc_sb")
            nc.scalar.copy(rc_sb[0:NSH, :], rc[:])
            nc.sync.dma_start(recon_d[:], rc_sb[0:NSH, :])

    _dedupe_ldweights(nc)
    nc.compile()
    return nc


def _get_program():
    if "nc" not in _cache:
        _cache["nc"] = _build_program()
    return _cache["nc"]


def _prep_inputs(q, K, V, codebook, Wq, Wc, bc, Wr, br):
    f16 = np.float16
    cbT = np.ascontiguousarray(codebook.T).astype(f16)
    wqT = np.ascontiguousarray(Wq.T).astype(f16)
    wcT = np.ascontiguousarray(Wc.T).astype(f16)
    wrT = np.ascontiguousarray(Wr.T).astype(f16)
    bc2 = bc.reshape(1, CLASSES).astype(f16)
    br2 = br.reshape(1, D).astype(f16)
    onesv = np.ones((1, P), dtype=f16)
    in_maps = []
    for c in range(NCORES):
        sl = slice(c * NSH, (c + 1) * NSH)
        qT = np.ascontiguousarray(q[sl].T).astype(f16)
        kT = np.ascontiguousarray(K[sl].reshape(NS, D).T).astype(f16)
        v16 = V[sl].astype(f16)
        in_maps.append(
            {
                "qT": qT,
                "kT": kT,
                "v": v16,
                "cbT": cbT,
                "wqT": wqT,
                "wcT": wcT,
                "wrT": wrT,
                "bc": bc2,
                "br": br2,
                "onesv": onesv,
            }
        )
    return in_maps


def kernel(q, K, V, codebook, Wq, Wc, bc, Wr, br, _trace=False):
    from concourse.bass_utils import run_bass_kernel_spmd

    q = np.asarray(q, dtype=np.float32)
    K = np.asarray(K, dtype=np.float32)
    V = np.asarray(V, dtype=np.float32)
    codebook = np.asarray(codebook, dtype=np.float32)
    Wq = np.asarray(Wq, dtype=np.float32)
    Wc = np.asarray(Wc, dtype=np.float32)
    bc = np.asarray(bc, dtype=np.float32)
    Wr = np.asarray(Wr, dtype=np.float32)
    br = np.asarray(br, dtype=np.float32)

    _patch_ldw_opt()
    nc = _get_program()
    in_maps = _prep_inputs(q, K, V, codebook, Wq, Wc, bc, Wr, br)
    res = run_bass_kernel_spmd(nc, in_maps, list(range(NCORES)), trace=_trace)
    logits = np.concatenate([res.results[c]["logits"] for c in range(NCORES)], axis=0)
    recon = np.concatenate([res.results[c]["recon"] for c in range(NCORES)], axis=0)
    eff = np.concatenate([res.results[c]["eff"] for c in range(NCORES)], axis=0)
    if _trace:
        return (logits, recon, eff), res
    return (logits, recon, eff)


# revision 49
# speedup vs baseline: 1.0205x; 1.0205x over previous
"""Trainium2 Bass kernel for CodebookRemapReadout.

Data-parallel over n across 8 NeuronCores. Each core handles 64 queries.

Math (per query n):
  q_proj      = q @ Wq.T
  cwu[c]      = exp(q_proj . cb_c / sqrt(D))          (unnormalized)
  Zq          = sum_c cwu[c]
  E[c, s]     = exp(K[n,s] . cb_c / sqrt(D))          (unnormalized)
  Z[s]        = sum_c E[c, s]
  W[s]        = sum_c cwu[c] E[c, s]
  eff[s]      = W[s] / (Zq * Z[s])                    (== softmax-contracted weights)
  summary     = eff @ V[n]
  logits      = summary @ Wc.T + bc
  recon       = summary @ Wr.T + br

Layout strategy: keep the feature/codebook dim on SBUF partitions everywhere
(host pre-transposes K, cb, Wq, Wc, Wr into fp16), so the big [NS, C] logits
matmul, the Z/W reduction and the final projections are all plain PE matmuls
with zero on-device input transposes.
"""

import math

import numpy as np

N, S, D, C, CLASSES = 512, 256, 512, 1024, 1000
NCORES = 8
NSH = N // NCORES          # 64 queries per core
P = 128                    # partitions
NS = NSH * S               # 16384 ns-columns per core
CHUNK = 512                # ns-columns per chunk (= 2 queries)
NCHUNK = NS // CHUNK       # 32
GROUP = 32                 # queries per summary group (32-aligned partition starts)
NGROUP = NSH // GROUP      # 4
SCALE = 1.0 / math.sqrt(D)

_cache = {}


def _patch_ldw_opt():
    """Flip walrus's --enable-ldw-opt to true (dedupe/overlap weight loads).
    Env-gated for A/B: set KERNEL_NO_LDW_OPT=1 to keep the default."""
    import os

    # walrus rejects Tile-emitted InstLdweights under ldw-opt ("InstLdweights
    # is not compatible with LDW optimization"), so this stays off unless
    # explicitly requested for experiments.
    if not os.environ.get("KERNEL_LDW_OPT"):
        return
    import concourse.bass_utils as bu

    if getattr(bu, "_ldw_opt_patched", False):
        return
    orig = bu.run_command
    def _run(cmd, *a, **kw):
        cmd = [
            "--enable-ldw-opt=true" if c == "--enable-ldw-opt=false" else c
            for c in cmd
        ]
        return orig(cmd, *a, **kw)
    bu.run_command = _run
    bu._ldw_opt_patched = True


def _dedupe_ldweights(nc):
    """Delete a Ldweights instruction when the immediately-previous Ldweights
    in the same block loads identical weights (same memref/offset/ap/mode)
    and the candidate carries no semaphore waits/updates. The PE array keeps
    the stationary operand across matmuls, so the reload is pure overhead
    (verified bit-exact on hardware). All matmuls here are fp16, which lower
    to explicit Ldweights+Matmult pairs — fp32 self-loading matmuls would
    clobber the array and are not used."""

    def sig_of(ins):
        ap0 = ins.ins[0]
        return (
            str(getattr(ap0, "memref", None)),
            getattr(ap0, "offset", None),
            str(getattr(ap0, "ap", None)),
            str(getattr(ins, "tile_position", None)),
            str(getattr(ins, "perf_mode", None)),
            str(getattr(ins, "is_transpose", None)),
        )

    removed = 0
    for f in nc.m.functions:
        for blk in f.blocks:
            last_sig = None
            drop = []
            for idx, ins in enumerate(blk.instructions):
                tn = type(ins).__name__
                if tn == "InstLdweights":
                    sig = sig_of(ins)
                    si = ins.sync_info
                    clean = si is None or (
                        len(si.on_wait) == 0 and len(si.on_update) == 0
                    )
                    if sig == last_sig and clean:
                        drop.append(idx)
                    else:
                        last_sig = sig
                elif tn == "InstMatmult":
                    pass  # fp16 matmults do not touch the weight registers
                elif ins.engine is not None and str(ins.engine) == "EngineType.PE":
                    # unknown PE instruction: be conservative
                    last_sig = None
            for idx in reversed(drop):
                del blk.instructions[idx]
            removed += len(drop)
    return removed


def _build_program():
    import concourse.bass as bass
    import concourse.bacc as bacc
    import concourse.tile as tile
    import concourse.mybir as mybir

    f16 = mybir.dt.float16
    f32 = mybir.dt.float32
    f32r = mybir.dt.float32r
    Exp = mybir.ActivationFunctionType.Exp

    nc = bacc.Bacc("TRN2", target_bir_lowering=False, debug=False)

    # ---- per-core DRAM I/O ----
    qT_d = nc.dram_tensor("qT", [D, NSH], f16, kind="ExternalInput")
    kT_d = nc.dram_tensor("kT", [D, NS], f16, kind="ExternalInput")
    v_d = nc.dram_tensor("v", [NSH, S, D], f16, kind="ExternalInput")
    cbT_d = nc.dram_tensor("cbT", [D, C], f16, kind="ExternalInput")
    wqT_d = nc.dram_tensor("wqT", [D, D], f16, kind="ExternalInput")
    wcT_d = nc.dram_tensor("wcT", [D, CLASSES], f16, kind="ExternalInput")
    wrT_d = nc.dram_tensor("wrT", [D, D], f16, kind="ExternalInput")
    bc_d = nc.dram_tensor("bc", [1, CLASSES], f16, kind="ExternalInput")
    br_d = nc.dram_tensor("br", [1, D], f16, kind="ExternalInput")
    ones_d = nc.dram_tensor("onesv", [1, P], f16, kind="ExternalInput")

    logits_d = nc.dram_tensor("logits", [NSH, CLASSES], f32, kind="ExternalOutput")
    recon_d = nc.dram_tensor("recon", [NSH, D], f32, kind="ExternalOutput")
    eff_d = nc.dram_tensor("eff", [NSH, S], f32, kind="ExternalOutput")

    DK = D // P  # 4 contraction chunks
    CT = C // P  # 8 codebook tiles

    with tile.TileContext(nc) as tc:
        with (
            tc.tile_pool(name="resident", bufs=1) as rp,
            tc.tile_pool(name="kt", bufs=24) as ktp,
            tc.tile_pool(name="et", bufs=20) as etp,
            tc.tile_pool(name="vt", bufs=6) as vtp,
            tc.tile_pool(name="small", bufs=3) as smp,
            tc.tile_pool(name="psL", bufs=3, space="PSUM") as psL,
            tc.tile_pool(name="psZW", bufs=2, space="PSUM") as psZW,
            tc.tile_pool(name="psBC", bufs=2, space="PSUM") as psBC,
            tc.tile_pool(name="psS", bufs=1, space="PSUM") as psS,
        ):
            # ---------- PE warm-up ----------
            # ~5us of dummy matmuls while the input DMAs land: pushes the PE
            # HAM clock-gate to 8/8 before real work starts.
            warm = rp.tile([P, CHUNK], f16, tag="warm", name="warm")
            nc.vector.memset(warm[:], 0.0)
            wps = psS.tile([P, CHUNK], f32, tag="s", name="wps")
            for _ in range(24):
                nc.tensor.matmul(wps[:], warm[:, 0:P], warm[:], start=True, stop=True)

            # ---------- resident loads ----------
            cbT = [rp.tile([P, C], f16, tag=f"cbT{i}", name=f"cbT{i}") for i in range(DK)]
            for i in range(DK):
                nc.sync.dma_start(cbT[i][:], cbT_d[i * P : (i + 1) * P, :])
            # views of the codebook weights as [128, 128] lhsT slices
            cbW = [
                [cbT[i][:, ct * P : (ct + 1) * P] for ct in range(C // P)]
                for i in range(DK)
            ]
            wqT = [rp.tile([P, D], f16, tag=f"wqT{i}", name=f"wqT{i}") for i in range(DK)]
            for i in range(DK):
                nc.sync.dma_start(wqT[i][:], wqT_d[i * P : (i + 1) * P, :])
            wcT = [rp.tile([P, CLASSES], f16, tag=f"wcT{i}", name=f"wcT{i}") for i in range(DK)]
            for i in range(DK):
                nc.sync.dma_start(wcT[i][:], wcT_d[i * P : (i + 1) * P, :])
            wrT = [rp.tile([P, D], f16, tag=f"wrT{i}", name=f"wrT{i}") for i in range(DK)]
            for i in range(DK):
                nc.sync.dma_start(wrT[i][:], wrT_d[i * P : (i + 1) * P, :])
            qT = [rp.tile([P, NSH], f16, tag=f"qT{i}", name=f"qT{i}") for i in range(DK)]
            for i in range(DK):
                nc.sync.dma_start(qT[i][:], qT_d[i * P : (i + 1) * P, :])
            bc_sb = rp.tile([1, CLASSES], f16, tag="bc", name="bc")
            nc.sync.dma_start(bc_sb[:], bc_d[:])
            br_sb = rp.tile([1, D], f16, tag="br", name="br")
            nc.sync.dma_start(br_sb[:], br_d[:])
            ones_sb = rp.tile([1, P], f16, tag="ones", name="ones")
            nc.sync.dma_start(ones_sb[:], ones_d[:])

            # ---------- query path ----------
            # q_projT [e, n] = Wq @ q.T  (4 e-tiles)
            qp = [rp.tile([P, NSH], f16, tag=f"qp{e}", name=f"qp{e}") for e in range(DK)]
            for e in range(DK):
                ps = psS.tile([P, NSH], f32, tag="s", name="qps")
                for dk in range(DK):
                    nc.tensor.matmul(
                        ps[:],
                        wqT[dk][:, e * P : (e + 1) * P],
                        qT[dk][:],
                        start=(dk == 0),
                        stop=(dk == DK - 1),
                    )
                nc.vector.tensor_copy(qp[e][:], ps[:])

            # cwu^T [c, n] tiles with an extra ones column (col NSH): used as
            # the fused Z/W reduction weights.
            cwu = [rp.tile([P, NSH + 1], f16, tag=f"cwu{ct}", name=f"cwu{ct}") for ct in range(CT)]
            for ct in range(CT):
                ps = psS.tile([P, NSH], f32, tag="s", name="qps")
                for ek in range(DK):
                    nc.tensor.matmul(
                        ps[:],
                        cbW[ek][ct][:],
                        qp[ek][:],
                        start=(ek == 0),
                        stop=(ek == DK - 1),
                    )
                nc.scalar.activation(cwu[ct][:, 0:NSH], ps[:], Exp, scale=SCALE)
                nc.vector.memset(cwu[ct][:, NSH : NSH + 1], 1.0)

            # Zq [1, n] then Rq = 1/Zq. tile_position=(0, 64) lands the row on
            # partition 64 — the same partition the per-chunk 1/Z row lives
            # on, so the broadcast matmul needs no partition-hop DMA.
            zq = psS.tile([P, NSH], f32, tag="s", name="zq")
            for ct in range(CT):
                nc.tensor.matmul(
                    zq[NSH : NSH + 1, :],
                    cwu[ct][:, NSH : NSH + 1],
                    cwu[ct][:, 0:NSH],
                    start=(ct == 0),
                    stop=(ct == CT - 1),
                    tile_position=(0, NSH),
                )
            rq32 = rp.tile([P, NSH], f32, tag="rq32", name="rq32")
            nc.vector.reciprocal(rq32[NSH : NSH + 1, :], zq[NSH : NSH + 1, :])
            rq16 = rp.tile([P, NSH], f16, tag="rq16", name="rq16")
            nc.vector.tensor_copy(rq16[NSH : NSH + 1, :], rq32[NSH : NSH + 1, :])

            # ---------- persistent staging ----------
            estg = rp.tile([P, S], f16, tag="estg", name="estg")         # eff fp16 [n, s]
            sstg = rp.tile([NSH, D], f16, tag="sstg", name="sstg")       # summary fp16 [n, d]

            # ---------- main loop: chunk PAIRS share stationary weights ----------
            # Per pair, the two chunks' matmuls are interleaved (dk-major) so
            # consecutive PE matmuls use identical weights; the post-pass
            # below then deletes the redundant Ldweights.
            pending = []
            for pr in range(NCHUNK // 2):
                kt2 = []
                for cc in range(2):
                    ch = 2 * pr + cc
                    ktc = [
                        ktp.tile([P, CHUNK], f16, tag="kt", name=f"kt{cc}_{i}")
                        for i in range(DK)
                    ]
                    for i in range(DK):
                        nc.sync.dma_start(
                            ktc[i][:],
                            kT_d[i * P : (i + 1) * P, ch * CHUNK : (ch + 1) * CHUNK],
                        )
                    kt2.append(ktc)
                # logits tiles -> exp
                et2 = [[], []]
                for ct in range(CT):
                    L2 = [
                        psL.tile([P, CHUNK], f32, tag="L", name=f"L{cc}")
                        for cc in range(2)
                    ]
                    for dk in range(DK):
                        for cc in range(2):
                            nc.tensor.matmul(
                                L2[cc][:],
                                cbW[dk][ct][:],
                                kt2[cc][dk][:],
                                start=(dk == 0),
                                stop=(dk == DK - 1),
                            )
                    for cc in range(2):
                        e_t = etp.tile([P, CHUNK], f16, tag="E", name="E")
                        nc.scalar.activation(e_t[:], L2[cc][:], Exp, scale=SCALE)
                        et2[cc].append(e_t)
                # deferred eff chains of the previous pair go here: their PE
                # matmuls sit behind this pair's logits matmuls, and before
                # this pair's Z/W matmuls (whose PSUM slots they release).
                def eff_chain(zw, n0):
                    # rz = 1/Z on row NSH, hop to partition 0
                    rz32 = smp.tile([P, CHUNK], f32, tag="rz32", name="rz32")
                    nc.vector.reciprocal(rz32[NSH : NSH + 1, :], zw[NSH : NSH + 1, :])
                    rz16 = smp.tile([P, CHUNK], f16, tag="rz16", name="rz16")
                    nc.vector.tensor_copy(
                        rz16[NSH : NSH + 1, :], rz32[NSH : NSH + 1, :]
                    )
                    # bcast[n, col] = Rq[n] * rz[col] — both operands on
                    # partition 64 (K=1 contraction on PE row-group 2)
                    bc_ps = psBC.tile([NSH, CHUNK], f32, tag="bc", name="bc_ps")
                    nc.tensor.matmul(
                        bc_ps[:],
                        rq16[NSH : NSH + 1, 0:NSH],
                        rz16[NSH : NSH + 1, :],
                        start=True,
                        stop=True,
                    )
                    # eff rows (fp32): multiply the whole 64-row block (engines
                    # need 32-aligned partition starts); rows n0, n0+1 valid.
                    # DVE reads at most one PSUM operand: stage bcast in SBUF.
                    bcs = smp.tile([P, CHUNK], f32, tag="bcs", name="bcs")
                    nc.vector.tensor_copy(bcs[0:NSH, :], bc_ps[:])
                    effc = smp.tile([P, CHUNK], f32, tag="effc", name="effc")
                    nc.vector.tensor_mul(effc[0:NSH, :], zw[0:NSH, :], bcs[0:NSH, :])
                    # fp16 staging ([n, s]); SWDGE DMA casts f32->f16
                    nc.gpsimd.dma_start(estg[n0 : n0 + 1, :], effc[n0 : n0 + 1, 0:S])
                    nc.gpsimd.dma_start(
                        estg[n0 + 1 : n0 + 2, :], effc[n0 + 1 : n0 + 2, S:]
                    )

                for args in pending:
                    eff_chain(*args)
                pending = []

                # fused Z/W: rows 0..NSH-1 = W per local query, row NSH = Z
                # (ct-major, chunk-inner so consecutive matmuls share weights)
                zw2 = [
                    psZW.tile([NSH + 1, CHUNK], f32, tag="zw", name=f"zw{cc}")
                    for cc in range(2)
                ]
                for ct in range(CT):
                    for cc in range(2):
                        nc.tensor.matmul(
                            zw2[cc][:],
                            cwu[ct][:, 0 : NSH + 1],
                            et2[cc][ct][:],
                            start=(ct == 0),
                            stop=(ct == CT - 1),
                        )
                for cc in range(2):
                    pending.append((zw2[cc], 2 * (2 * pr + cc)))
                if pr == NCHUNK // 2 - 1:
                    for args in pending:
                        eff_chain(*args)
                    pending = []

                # ---------- summary for a finished group ----------
                # deferred one pair past the group's last chunk so the
                # transpose + matmuls never stall the PE pipeline
                PPG = NCHUNK // 2 // NGROUP  # pairs per group
                if (pr >= PPG and (pr - PPG) % PPG == 0) or pr == NCHUNK // 2 - 1:
                    g = (pr - PPG) // PPG if pr != NCHUNK // 2 - 1 else NGROUP - 1
                    gn = g * GROUP
                    efft_a = smp.tile([P, GROUP], f16, tag="efft_a", name="efft_a")
                    nc.sync.dma_start_transpose(
                        out=efft_a[:], in_=estg[gn : gn + GROUP, 0:P]
                    )
                    efft_b = smp.tile([P, GROUP], f16, tag="efft_b", name="efft_b")
                    nc.sync.dma_start_transpose(
                        out=efft_b[:], in_=estg[gn : gn + GROUP, P:S]
                    )
                    for j in range(GROUP):
                        n = gn + j
                        v_t = vtp.tile([P, 2, D], f16, tag="v", name="v_t")
                        nc.sync.dma_start(
                            v_t[:], v_d[n].rearrange("(a p) d -> p a d", p=P)
                        )
                        s_ps = psS.tile([1, D], f32, tag="s", name="s_ps")
                        nc.tensor.matmul(
                            s_ps[:],
                            efft_a[:, j : j + 1],
                            v_t[:, 0, :],
                            start=True,
                            stop=False,
                        )
                        nc.tensor.matmul(
                            s_ps[:],
                            efft_b[:, j : j + 1],
                            v_t[:, 1, :],
                            start=False,
                            stop=True,
                        )
                        srow = smp.tile([P, D], f16, tag="srow", name="srow")
                        if j % 2 == 0:
                            nc.vector.tensor_copy(srow[0:1, :], s_ps[:])
                        else:
                            nc.scalar.copy(srow[0:1, :], s_ps[:])
                        nc.sync.dma_start(sstg[n : n + 1, :], srow[0:1, :])

            # eff output in one shot (SWDGE casts the staged fp16 back to f32)
            nc.gpsimd.dma_start(eff_d[:], estg[0:NSH, :])

            # ---------- final projections ----------
            st = [rp.tile([P, NSH], f16, tag=f"st{i}", name=f"st{i}") for i in range(DK)]
            for i in range(DK):
                nc.sync.dma_start_transpose(
                    out=st[i][:], in_=sstg[:, i * P : (i + 1) * P]
                )
            # logits = summary @ Wc.T + bc   (two 500-wide halves)
            H = CLASSES // 2
            for h in range(2):
                lg = psBC.tile([NSH, H], f32, tag="bc", name="lg")
                nc.tensor.matmul(
                    lg[:],
                    ones_sb[0:1, 0:NSH],
                    bc_sb[0:1, h * H : (h + 1) * H],
                    start=True,
                    stop=False,
                )
                for dk in range(DK):
                    nc.tensor.matmul(
                        lg[:],
                        st[dk][:, 0:NSH],
                        wcT[dk][:, h * H : (h + 1) * H],
                        start=False,
                        stop=(dk == DK - 1),
                    )
                lg_sb = smp.tile([P, H], f32, tag="lg_sb", name="lg_sb")
                nc.scalar.copy(lg_sb[0:NSH, :], lg[:])
                nc.sync.dma_start(logits_d[:, h * H : (h + 1) * H], lg_sb[0:NSH, :])
            # recon = summary @ Wr.T + br
            rc = psBC.tile([NSH, D], f32, tag="bc", name="rc")
            nc.tensor.matmul(
                rc[:], ones_sb[0:1, 0:NSH], br_sb[0:1, :], start=True, stop=False
            )
            for dk in range(DK):
                nc.tensor.matmul(
                    rc[:],
                    st[dk][:, 0:NSH],
                    wrT[dk][:, :],
                    start=False,
                    stop=(dk == DK - 1),
                )
            rc_sb = smp.tile([P, D], f32, tag="rc_sb", name="rc_sb")
            nc.scalar.copy(rc_sb[0:NSH, :], rc[:])
            nc.sync.dma_start(recon_d[:], rc_sb[0:NSH, :])

    _dedupe_ldweights(nc)
    nc.compile()
    return nc


def _get_program():
    if "nc" not in _cache:
        _cache["nc"] = _build_program()
    return _cache["nc"]


def _prep_inputs(q, K, V, codebook, Wq, Wc, bc, Wr, br):
    f16 = np.float16
    cbT = np.ascontiguousarray(codebook.T).astype(f16)
    wqT = np.ascontiguousarray(Wq.T).astype(f16)
    wcT = np.ascontiguousarray(Wc.T).astype(f16)
    wrT = np.ascontiguousarray(Wr.T).astype(f16)
    bc2 = bc.reshape(1, CLASSES).astype(f16)
    br2 = br.reshape(1, D).astype(f16)
    onesv = np.ones((1, P), dtype=f16)
    in_maps = []
    for c in range(NCORES):
        sl = slice(c * NSH, (c + 1) * NSH)
        qT = np.ascontiguousarray(q[sl].T).astype(f16)
        kT = np.ascontiguousarray(K[sl].reshape(NS, D).T).astype(f16)
        v16 = V[sl].astype(f16)
        in_maps.append(
            {
                "qT": qT,
                "kT": kT,
                "v": v16,
                "cbT": cbT,
                "wqT": wqT,
                "wcT": wcT,
                "wrT": wrT,
                "bc": bc2,
                "br": br2,
                "onesv": onesv,
            }
        )
    return in_maps


def kernel(q, K, V, codebook, Wq, Wc, bc, Wr, br, _trace=False):
    from concourse.bass_utils import run_bass_kernel_spmd

    q = np.asarray(q, dtype=np.float32)
    K = np.asarray(K, dtype=np.float32)
    V = np.asarray(V, dtype=np.float32)
    codebook = np.asarray(codebook, dtype=np.float32)
    Wq = np.asarray(Wq, dtype=np.float32)
    Wc = np.asarray(Wc, dtype=np.float32)
    bc = np.asarray(bc, dtype=np.float32)
    Wr = np.asarray(Wr, dtype=np.float32)
    br = np.asarray(br, dtype=np.float32)

    _patch_ldw_opt()
    nc = _get_program()
    in_maps = _prep_inputs(q, K, V, codebook, Wq, Wc, bc, Wr, br)
    res = run_bass_kernel_spmd(nc, in_maps, list(range(NCORES)), trace=_trace)
    logits = np.concatenate([res.results[c]["logits"] for c in range(NCORES)], axis=0)
    recon = np.concatenate([res.results[c]["recon"] for c in range(NCORES)], axis=0)
    eff = np.concatenate([res.results[c]["eff"] for c in range(NCORES)], axis=0)
    if _trace:
        return (logits, recon, eff), res
    return (logits, recon, eff)


# revision 52
# speedup vs baseline: 1.0321x; 1.0113x over previous
"""Trainium2 Bass kernel for CodebookRemapReadout.

Data-parallel over n across 8 NeuronCores. Each core handles 64 queries.

Math (per query n):
  q_proj      = q @ Wq.T
  cwu[c]      = exp(q_proj . cb_c / sqrt(D))          (unnormalized)
  Zq          = sum_c cwu[c]
  E[c, s]     = exp(K[n,s] . cb_c / sqrt(D))          (unnormalized)
  Z[s]        = sum_c E[c, s]
  W[s]        = sum_c cwu[c] E[c, s]
  eff[s]      = W[s] / (Zq * Z[s])                    (== softmax-contracted weights)
  summary     = eff @ V[n]
  logits      = summary @ Wc.T + bc
  recon       = summary @ Wr.T + br

Layout strategy: keep the feature/codebook dim on SBUF partitions everywhere
(host pre-transposes K, cb, Wq, Wc, Wr into fp16), so the big [NS, C] logits
matmul, the Z/W reduction and the final projections are all plain PE matmuls
with zero on-device input transposes.
"""

import math

import numpy as np

N, S, D, C, CLASSES = 512, 256, 512, 1024, 1000
NCORES = 8
NSH = N // NCORES          # 64 queries per core
P = 128                    # partitions
NS = NSH * S               # 16384 ns-columns per core
CHUNK = 512                # ns-columns per chunk (= 2 queries)
NCHUNK = NS // CHUNK       # 32
GROUP = 32                 # queries per summary group (32-aligned partition starts)
NGROUP = NSH // GROUP      # 4
SCALE = 1.0 / math.sqrt(D)

_cache = {}


def _patch_ldw_opt():
    """Flip walrus's --enable-ldw-opt to true (dedupe/overlap weight loads).
    Env-gated for A/B: set KERNEL_NO_LDW_OPT=1 to keep the default."""
    import os

    # walrus rejects Tile-emitted InstLdweights under ldw-opt ("InstLdweights
    # is not compatible with LDW optimization"), so this stays off unless
    # explicitly requested for experiments.
    if not os.environ.get("KERNEL_LDW_OPT"):
        return
    import concourse.bass_utils as bu

    if getattr(bu, "_ldw_opt_patched", False):
        return
    orig = bu.run_command
    def _run(cmd, *a, **kw):
        cmd = [
            "--enable-ldw-opt=true" if c == "--enable-ldw-opt=false" else c
            for c in cmd
        ]
        return orig(cmd, *a, **kw)
    bu.run_command = _run
    bu._ldw_opt_patched = True


def _dedupe_ldweights(nc):
    """Delete a Ldweights instruction when the immediately-previous Ldweights
    in the same block loads identical weights (same memref/offset/ap/mode)
    and the candidate carries no semaphore waits/updates. The PE array keeps
    the stationary operand across matmuls, so the reload is pure overhead
    (verified bit-exact on hardware). All matmuls here are fp16, which lower
    to explicit Ldweights+Matmult pairs — fp32 self-loading matmuls would
    clobber the array and are not used."""

    def sig_of(ins):
        ap0 = ins.ins[0]
        return (
            str(getattr(ap0, "memref", None)),
            getattr(ap0, "offset", None),
            str(getattr(ap0, "ap", None)),
            str(getattr(ins, "tile_position", None)),
            str(getattr(ins, "perf_mode", None)),
            str(getattr(ins, "is_transpose", None)),
        )

    removed = 0
    for f in nc.m.functions:
        for blk in f.blocks:
            last_sig = None
            drop = []
            for idx, ins in enumerate(blk.instructions):
                tn = type(ins).__name__
                if tn == "InstLdweights":
                    sig = sig_of(ins)
                    si = ins.sync_info
                    clean = si is None or (
                        len(si.on_wait) == 0 and len(si.on_update) == 0
                    )
                    if sig == last_sig and clean:
                        drop.append(idx)
                    else:
                        last_sig = sig
                elif tn == "InstMatmult":
                    pass  # fp16 matmults do not touch the weight registers
                elif ins.engine is not None and str(ins.engine) == "EngineType.PE":
                    # unknown PE instruction: be conservative
                    last_sig = None
            for idx in reversed(drop):
                del blk.instructions[idx]
            removed += len(drop)
    return removed


def _build_program():
    import concourse.bass as bass
    import concourse.bacc as bacc
    import concourse.tile as tile
    import concourse.mybir as mybir

    f16 = mybir.dt.float16
    f32 = mybir.dt.float32
    f32r = mybir.dt.float32r
    Exp = mybir.ActivationFunctionType.Exp

    nc = bacc.Bacc("TRN2", target_bir_lowering=False, debug=False)

    # ---- per-core DRAM I/O ----
    qT_d = nc.dram_tensor("qT", [D, NSH], f16, kind="ExternalInput")
    kT_d = nc.dram_tensor("kT", [D, NS], f16, kind="ExternalInput")
    v_d = nc.dram_tensor("v", [NSH, S, D], f16, kind="ExternalInput")
    cbT_d = nc.dram_tensor("cbT", [D, C], f16, kind="ExternalInput")
    wqT_d = nc.dram_tensor("wqT", [D, D], f16, kind="ExternalInput")
    wcT_d = nc.dram_tensor("wcT", [D, CLASSES], f16, kind="ExternalInput")
    wrT_d = nc.dram_tensor("wrT", [D, D], f16, kind="ExternalInput")
    bc_d = nc.dram_tensor("bc", [1, CLASSES], f16, kind="ExternalInput")
    br_d = nc.dram_tensor("br", [1, D], f16, kind="ExternalInput")
    ones_d = nc.dram_tensor("onesv", [1, P], f16, kind="ExternalInput")

    logits_d = nc.dram_tensor("logits", [NSH, CLASSES], f32, kind="ExternalOutput")
    recon_d = nc.dram_tensor("recon", [NSH, D], f32, kind="ExternalOutput")
    eff_d = nc.dram_tensor("eff", [NSH, S], f32, kind="ExternalOutput")

    DK = D // P  # 4 contraction chunks
    CT = C // P  # 8 codebook tiles

    with tile.TileContext(nc) as tc:
        with (
            tc.tile_pool(name="resident", bufs=1) as rp,
            tc.tile_pool(name="kt", bufs=24) as ktp,
            tc.tile_pool(name="et", bufs=20) as etp,
            tc.tile_pool(name="vt", bufs=6) as vtp,
            tc.tile_pool(name="small", bufs=3) as smp,
            tc.tile_pool(name="psL", bufs=3, space="PSUM") as psL,
            tc.tile_pool(name="psZW", bufs=2, space="PSUM") as psZW,
            tc.tile_pool(name="psBC", bufs=1, space="PSUM") as psBC,
            tc.tile_pool(name="psS", bufs=2, space="PSUM") as psS,
        ):
            # ---------- PE warm-up ----------
            # ~5us of dummy matmuls while the input DMAs land: pushes the PE
            # HAM clock-gate to 8/8 before real work starts.
            warm = rp.tile([P, CHUNK], f16, tag="warm", name="warm")
            nc.vector.memset(warm[:], 0.0)
            wps = psS.tile([P, CHUNK], f32, tag="s", name="wps")
            for _ in range(24):
                nc.tensor.matmul(wps[:], warm[:, 0:P], warm[:], start=True, stop=True)

            # ---------- resident loads (query-critical ones first) ----------
            qT = [rp.tile([P, NSH], f16, tag=f"qT{i}", name=f"qT{i}") for i in range(DK)]
            for i in range(DK):
                nc.sync.dma_start(qT[i][:], qT_d[i * P : (i + 1) * P, :])
            wqT = [rp.tile([P, D], f16, tag=f"wqT{i}", name=f"wqT{i}") for i in range(DK)]
            for i in range(DK):
                nc.sync.dma_start(wqT[i][:], wqT_d[i * P : (i + 1) * P, :])
            cbT = [rp.tile([P, C], f16, tag=f"cbT{i}", name=f"cbT{i}") for i in range(DK)]
            for i in range(DK):
                nc.sync.dma_start(cbT[i][:], cbT_d[i * P : (i + 1) * P, :])
            # views of the codebook weights as [128, 128] lhsT slices
            cbW = [
                [cbT[i][:, ct * P : (ct + 1) * P] for ct in range(C // P)]
                for i in range(DK)
            ]

            # ---------- query path ----------
            # q_projT [e, n] = Wq @ q.T  (4 e-tiles)
            qp = [rp.tile([P, NSH], f16, tag=f"qp{e}", name=f"qp{e}") for e in range(DK)]
            for e in range(DK):
                ps = psS.tile([P, NSH], f32, tag="s", name="qps")
                for dk in range(DK):
                    nc.tensor.matmul(
                        ps[:],
                        wqT[dk][:, e * P : (e + 1) * P],
                        qT[dk][:],
                        start=(dk == 0),
                        stop=(dk == DK - 1),
                    )
                nc.vector.tensor_copy(qp[e][:], ps[:])

            # cwu^T [c, n] tiles with an extra ones column (col NSH): used as
            # the fused Z/W reduction weights.
            cwu = [rp.tile([P, NSH + 1], f16, tag=f"cwu{ct}", name=f"cwu{ct}") for ct in range(CT)]
            for ct in range(CT):
                ps = psS.tile([P, NSH], f32, tag="s", name="qps")
                for ek in range(DK):
                    nc.tensor.matmul(
                        ps[:],
                        cbW[ek][ct][:],
                        qp[ek][:],
                        start=(ek == 0),
                        stop=(ek == DK - 1),
                    )
                nc.scalar.activation(cwu[ct][:, 0:NSH], ps[:], Exp, scale=SCALE)
                nc.vector.memset(cwu[ct][:, NSH : NSH + 1], 1.0)

            # Zq [1, n] then Rq = 1/Zq. tile_position=(0, 64) lands the row on
            # partition 64 — the same partition the per-chunk 1/Z row lives
            # on, so the broadcast matmul needs no partition-hop DMA.
            zq = psS.tile([P, NSH], f32, tag="s", name="zq")
            for ct in range(CT):
                nc.tensor.matmul(
                    zq[NSH : NSH + 1, :],
                    cwu[ct][:, NSH : NSH + 1],
                    cwu[ct][:, 0:NSH],
                    start=(ct == 0),
                    stop=(ct == CT - 1),
                    tile_position=(0, NSH),
                )
            rq32 = rp.tile([P, NSH], f32, tag="rq32", name="rq32")
            nc.vector.reciprocal(rq32[NSH : NSH + 1, :], zq[NSH : NSH + 1, :])
            rq16 = rp.tile([P, NSH], f16, tag="rq16", name="rq16")
            nc.vector.tensor_copy(rq16[NSH : NSH + 1, :], rq32[NSH : NSH + 1, :])

            # projection weights are only needed at the very end; load them
            # after the query path so they don't delay the first K chunks
            wcT = [rp.tile([P, CLASSES], f16, tag=f"wcT{i}", name=f"wcT{i}") for i in range(DK)]
            for i in range(DK):
                nc.sync.dma_start(wcT[i][:], wcT_d[i * P : (i + 1) * P, :])
            wrT = [rp.tile([P, D], f16, tag=f"wrT{i}", name=f"wrT{i}") for i in range(DK)]
            for i in range(DK):
                nc.sync.dma_start(wrT[i][:], wrT_d[i * P : (i + 1) * P, :])
            bc_sb = rp.tile([1, CLASSES], f16, tag="bc", name="bc")
            nc.sync.dma_start(bc_sb[:], bc_d[:])
            br_sb = rp.tile([1, D], f16, tag="br", name="br")
            nc.sync.dma_start(br_sb[:], br_d[:])
            ones_sb = rp.tile([1, P], f16, tag="ones", name="ones")
            nc.sync.dma_start(ones_sb[:], ones_d[:])

            # ---------- persistent staging ----------
            estg = rp.tile([P, S], f16, tag="estg", name="estg")         # eff fp16 [n, s]
            sstg = rp.tile([NSH, D], f16, tag="sstg", name="sstg")       # summary fp16 [n, d]

            # ---------- main loop: chunk PAIRS share stationary weights ----------
            # Per pair, the two chunks' matmuls are interleaved (dk-major) so
            # consecutive PE matmuls use identical weights; the post-pass
            # below then deletes the redundant Ldweights.
            pending = []
            for pr in range(NCHUNK // 2):
                kt2 = []
                for cc in range(2):
                    ch = 2 * pr + cc
                    ktc = [
                        ktp.tile([P, CHUNK], f16, tag="kt", name=f"kt{cc}_{i}")
                        for i in range(DK)
                    ]
                    for i in range(DK):
                        nc.sync.dma_start(
                            ktc[i][:],
                            kT_d[i * P : (i + 1) * P, ch * CHUNK : (ch + 1) * CHUNK],
                        )
                    kt2.append(ktc)
                # logits tiles -> exp
                et2 = [[], []]
                for ct in range(CT):
                    L2 = [
                        psL.tile([P, CHUNK], f32, tag="L", name=f"L{cc}")
                        for cc in range(2)
                    ]
                    for dk in range(DK):
                        for cc in range(2):
                            nc.tensor.matmul(
                                L2[cc][:],
                                cbW[dk][ct][:],
                                kt2[cc][dk][:],
                                start=(dk == 0),
                                stop=(dk == DK - 1),
                            )
                    for cc in range(2):
                        e_t = etp.tile([P, CHUNK], f16, tag="E", name="E")
                        nc.scalar.activation(e_t[:], L2[cc][:], Exp, scale=SCALE)
                        et2[cc].append(e_t)
                # deferred eff chains of the previous pair go here: their PE
                # matmuls sit behind this pair's logits matmuls, and before
                # this pair's Z/W matmuls (whose PSUM slots they release).
                def eff_chain(zw, n0):
                    # rz = 1/Z on row NSH, hop to partition 0
                    rz32 = smp.tile([P, CHUNK], f32, tag="rz32", name="rz32")
                    nc.vector.reciprocal(rz32[NSH : NSH + 1, :], zw[NSH : NSH + 1, :])
                    rz16 = smp.tile([P, CHUNK], f16, tag="rz16", name="rz16")
                    nc.vector.tensor_copy(
                        rz16[NSH : NSH + 1, :], rz32[NSH : NSH + 1, :]
                    )
                    # bcast[n, col] = Rq[n] * rz[col] — both operands on
                    # partition 64 (K=1 contraction on PE row-group 2)
                    bc_ps = psBC.tile([NSH, CHUNK], f32, tag="bc", name="bc_ps")
                    nc.tensor.matmul(
                        bc_ps[:],
                        rq16[NSH : NSH + 1, 0:NSH],
                        rz16[NSH : NSH + 1, :],
                        start=True,
                        stop=True,
                    )
                    # eff rows (fp32): multiply the whole 64-row block (engines
                    # need 32-aligned partition starts); rows n0, n0+1 valid.
                    # DVE reads at most one PSUM operand: stage bcast in SBUF.
                    bcs = smp.tile([P, CHUNK], f32, tag="bcs", name="bcs")
                    nc.vector.tensor_copy(bcs[0:NSH, :], bc_ps[:])
                    effc = smp.tile([P, CHUNK], f32, tag="effc", name="effc")
                    nc.vector.tensor_mul(effc[0:NSH, :], zw[0:NSH, :], bcs[0:NSH, :])
                    # fp16 staging ([n, s]); SWDGE DMA casts f32->f16
                    nc.gpsimd.dma_start(estg[n0 : n0 + 1, :], effc[n0 : n0 + 1, 0:S])
                    nc.gpsimd.dma_start(
                        estg[n0 + 1 : n0 + 2, :], effc[n0 + 1 : n0 + 2, S:]
                    )

                for args in pending:
                    eff_chain(*args)
                pending = []

                # fused Z/W: rows 0..NSH-1 = W per local query, row NSH = Z
                # (ct-major, chunk-inner so consecutive matmuls share weights)
                zw2 = [
                    psZW.tile([NSH + 1, CHUNK], f32, tag="zw", name=f"zw{cc}")
                    for cc in range(2)
                ]
                for ct in range(CT):
                    for cc in range(2):
                        nc.tensor.matmul(
                            zw2[cc][:],
                            cwu[ct][:, 0 : NSH + 1],
                            et2[cc][ct][:],
                            start=(ct == 0),
                            stop=(ct == CT - 1),
                        )
                for cc in range(2):
                    pending.append((zw2[cc], 2 * (2 * pr + cc)))
                if pr == NCHUNK // 2 - 1:
                    for args in pending:
                        eff_chain(*args)
                    pending = []

                # ---------- summary for a finished group ----------
                # deferred one pair past the group's last chunk so the
                # transpose + matmuls never stall the PE pipeline
                PPG = NCHUNK // 2 // NGROUP  # pairs per group
                if (pr >= PPG and (pr - PPG) % PPG == 0) or pr == NCHUNK // 2 - 1:
                    g = (pr - PPG) // PPG if pr != NCHUNK // 2 - 1 else NGROUP - 1
                    gn = g * GROUP
                    efft_a = smp.tile([P, GROUP], f16, tag="efft_a", name="efft_a")
                    nc.sync.dma_start_transpose(
                        out=efft_a[:], in_=estg[gn : gn + GROUP, 0:P]
                    )
                    efft_b = smp.tile([P, GROUP], f16, tag="efft_b", name="efft_b")
                    nc.sync.dma_start_transpose(
                        out=efft_b[:], in_=estg[gn : gn + GROUP, P:S]
                    )
                    for j in range(GROUP):
                        n = gn + j
                        v_t = vtp.tile([P, 2, D], f16, tag="v", name="v_t")
                        nc.sync.dma_start(
                            v_t[:], v_d[n].rearrange("(a p) d -> p a d", p=P)
                        )
                        s_ps = psS.tile([1, D], f32, tag="s", name="s_ps")
                        nc.tensor.matmul(
                            s_ps[:],
                            efft_a[:, j : j + 1],
                            v_t[:, 0, :],
                            start=True,
                            stop=False,
                        )
                        nc.tensor.matmul(
                            s_ps[:],
                            efft_b[:, j : j + 1],
                            v_t[:, 1, :],
                            start=False,
                            stop=True,
                        )
                        srow = smp.tile([P, D], f16, tag="srow", name="srow")
                        if j % 2 == 0:
                            nc.vector.tensor_copy(srow[0:1, :], s_ps[:])
                        else:
                            nc.scalar.copy(srow[0:1, :], s_ps[:])
                        nc.sync.dma_start(sstg[n : n + 1, :], srow[0:1, :])

            # eff output in one shot (SWDGE casts the staged fp16 back to f32)
            nc.gpsimd.dma_start(eff_d[:], estg[0:NSH, :])

            # ---------- final projections ----------
            st = [rp.tile([P, NSH], f16, tag=f"st{i}", name=f"st{i}") for i in range(DK)]
            for i in range(DK):
                nc.sync.dma_start_transpose(
                    out=st[i][:], in_=sstg[:, i * P : (i + 1) * P]
                )
            # logits = summary @ Wc.T + bc   (two 500-wide halves)
            H = CLASSES // 2
            for h in range(2):
                lg = psBC.tile([NSH, H], f32, tag="bc", name="lg")
                nc.tensor.matmul(
                    lg[:],
                    ones_sb[0:1, 0:NSH],
                    bc_sb[0:1, h * H : (h + 1) * H],
                    start=True,
                    stop=False,
                )
                for dk in range(DK):
                    nc.tensor.matmul(
                        lg[:],
                        st[dk][:, 0:NSH],
                        wcT[dk][:, h * H : (h + 1) * H],
                        start=False,
                        stop=(dk == DK - 1),
                    )
                lg_sb = smp.tile([P, H], f32, tag="lg_sb", name="lg_sb")
                nc.scalar.copy(lg_sb[0:NSH, :], lg[:])
                nc.sync.dma_start(logits_d[:, h * H : (h + 1) * H], lg_sb[0:NSH, :])
            # recon = summary @ Wr.T + br
            rc = psBC.tile([NSH, D], f32, tag="bc", name="rc")
            nc.tensor.matmul(
                rc[:], ones_sb[0:1, 0:NSH], br_sb[0:1, :], start=True, stop=False
            )
            for dk in range(DK):
                nc.tensor.matmul(
                    rc[:],
                    st[dk][:, 0:NSH],
                    wrT[dk][:, :],
                    start=False,
                    stop=(dk == DK - 1),
                )
            rc_sb = smp.tile([P, D], f32, tag="rc_sb", name="rc_sb")
            nc.scalar.copy(rc_sb[0:NSH, :], rc[:])
            nc.sync.dma_start(recon_d[:], rc_sb[0:NSH, :])

    _dedupe_ldweights(nc)
    nc.compile()
    return nc


def _get_program():
    if "nc" not in _cache:
        _cache["nc"] = _build_program()
    return _cache["nc"]


def _prep_inputs(q, K, V, codebook, Wq, Wc, bc, Wr, br):
    f16 = np.float16
    cbT = np.ascontiguousarray(codebook.T).astype(f16)
    wqT = np.ascontiguousarray(Wq.T).astype(f16)
    wcT = np.ascontiguousarray(Wc.T).astype(f16)
    wrT = np.ascontiguousarray(Wr.T).astype(f16)
    bc2 = bc.reshape(1, CLASSES).astype(f16)
    br2 = br.reshape(1, D).astype(f16)
    onesv = np.ones((1, P), dtype=f16)
    in_maps = []
    for c in range(NCORES):
        sl = slice(c * NSH, (c + 1) * NSH)
        qT = np.ascontiguousarray(q[sl].T).astype(f16)
        kT = np.ascontiguousarray(K[sl].reshape(NS, D).T).astype(f16)
        v16 = V[sl].astype(f16)
        in_maps.append(
            {
                "qT": qT,
                "kT": kT,
                "v": v16,
                "cbT": cbT,
                "wqT": wqT,
                "wcT": wcT,
                "wrT": wrT,
                "bc": bc2,
                "br": br2,
                "onesv": onesv,
            }
        )
    return in_maps


def kernel(q, K, V, codebook, Wq, Wc, bc, Wr, br, _trace=False):
    from concourse.bass_utils import run_bass_kernel_spmd

    q = np.asarray(q, dtype=np.float32)
    K = np.asarray(K, dtype=np.float32)
    V = np.asarray(V, dtype=np.float32)
    codebook = np.asarray(codebook, dtype=np.float32)
    Wq = np.asarray(Wq, dtype=np.float32)
    Wc = np.asarray(Wc, dtype=np.float32)
    bc = np.asarray(bc, dtype=np.float32)
    Wr = np.asarray(Wr, dtype=np.float32)
    br = np.asarray(br, dtype=np.float32)

    _patch_ldw_opt()
    nc = _get_program()
    in_maps = _prep_inputs(q, K, V, codebook, Wq, Wc, bc, Wr, br)
    res = run_bass_kernel_spmd(nc, in_maps, list(range(NCORES)), trace=_trace)
    logits = np.concatenate([res.results[c]["logits"] for c in range(NCORES)], axis=0)
    recon = np.concatenate([res.results[c]["recon"] for c in range(NCORES)], axis=0)
    eff = np.concatenate([res.results[c]["eff"] for c in range(NCORES)], axis=0)
    if _trace:
        return (logits, recon, eff), res
    return (logits, recon, eff)
